# revision 1
# baseline (speedup 1.0000x reference)
"""Trainium2 Bass kernel for ABMIL-MoE-LoRA linear layer.

Reference computation (B=4, N=2048, D_IN=D_OUT=4096, E=8, R=16, D_ATT=128):
    base = x @ W.T + bias
    v = tanh(x @ V.T); u = sigmoid(x @ U.T)
    rw = sigmoid((v*u) @ router_W.T)                    # [B,N,E]
    lora = x @ A_e  (per expert)                        # [B,N,E,R]
    out = base + sum_e rw[...,e] * (lora_e @ B_e)

Strategy: data-parallel over the B*N = 8192 tokens across 8 NeuronCores
(1024 tokens/core, weights replicated). All matmuls run in bf16 on the
TensorEngine with fp32 PSUM accumulation. Host-side prep pre-transposes
every operand so the contraction dim lands on SBUF partitions.

Schedule: the router/LoRA-down projections are interleaved into the first
two output-column sweeps (k-tile by k-tile, matching DMA arrival order) so
the TensorEngine never starves while x / weights stream in. Those two
sweeps accumulate base-matmul partials into SBUF (PSUM banks are the
scarce resource); later sweeps use the classic 8-bank PSUM accumulation
with the MoE up-projection matmul fused into the same accumulation group.

Self-contained: hardcodes all shapes; only imports installed packages.
"""

import numpy as np
import ml_dtypes

BF16 = ml_dtypes.bfloat16

# Problem shapes (hardcoded per spec)
B, N, D_IN, D_OUT = 4, 2048, 4096, 4096
E, R, D_ATT = 8, 16, 128
TOKENS = B * N            # 8192
N_CORES = 8
T = TOKENS // N_CORES     # 1024 tokens per core
KT = D_IN // 128          # 32 contraction k-tiles
OC = 512                  # output-column chunk per PSUM bank
NOC = D_OUT // OC         # 8 o-chunks
TT = T // 128             # 8 token tiles per core
KH = 2                    # weight streamed in 2 k-halves
KHT = KT // KH            # 16 k-tiles per half

_CACHE = {}


def _get_nc():
    if "nc" in _CACHE:
        return _CACHE["nc"]

    import concourse.tile as tile
    import concourse.mybir as mybir
    from concourse import bacc

    dt = mybir.dt
    AFT = mybir.ActivationFunctionType
    nc = bacc.Bacc("TRN2", target_bir_lowering=False, debug=False)

    xT = nc.declare_dram_parameter("xT", [D_IN, T], dt.bfloat16, isOutput=False)
    wT = nc.declare_dram_parameter("wT", [D_IN, D_OUT], dt.bfloat16, isOutput=False)
    projT = nc.declare_dram_parameter("projT", [D_IN, 384], dt.float8e4, isOutput=False)
    xR8 = nc.declare_dram_parameter("xR8", [D_IN, T], dt.float8e4, isOutput=False)
    rwrep = nc.declare_dram_parameter("rwrep", [128, 128], dt.bfloat16, isOutput=False)
    bcat = nc.declare_dram_parameter("bcat", [E * R, D_OUT], dt.bfloat16, isOutput=False)
    biasr = nc.declare_dram_parameter("biasr", [128, D_OUT], dt.bfloat16, isOutput=False)
    out = nc.declare_dram_parameter("out", [T, D_OUT], dt.float32, isOutput=True)

    xT_ap, wT_ap, projT_ap, xR8_ap = xT.ap(), wT.ap(), projT.ap(), xR8.ap()
    rwrep_ap, bcat_ap, biasr_ap, out_ap = rwrep.ap(), bcat.ap(), biasr.ap(), out.ap()

    with tile.TileContext(nc) as tc:
        with (
            tc.tile_pool(name="xpool", bufs=1) as xpool,
            tc.tile_pool(name="wpool", bufs=2) as wpool,
            tc.tile_pool(name="w0pool", bufs=1) as w0pool,
            tc.tile_pool(name="const", bufs=1) as constp,
            tc.tile_pool(name="inter", bufs=1) as inter,
            tc.tile_pool(name="accpool", bufs=1) as accpool,
            tc.tile_pool(name="opool", bufs=3) as opool,
            tc.tile_pool(name="ps", bufs=8, space="PSUM") as psp,
        ):
            xsb = xpool.tile([128, KT * T], dt.bfloat16, tag="xsb")
            vub = inter.tile([128, T], dt.bfloat16, tag="vub")
            rwb = inter.tile([128, T], dt.bfloat16, tag="rwb")
            wtb = inter.tile([128, T], dt.bfloat16, tag="wtb")
            lsb0 = inter.tile([128, 512], dt.bfloat16, tag="lsb0")
            acc = {}  # (oc, t) -> SBUF fp32 partial-sum tile for sweeps 0/1

            def ps_tile(name):
                return psp.tile([128, 512], dt.float32, tag="ps", name=name)

            # ---- sweeps 0 and 1: router half-sweep h fused with the base
            # matmul for o-chunk 0, token-half h. The oc0 weight chunk stays
            # resident across both sweeps; each (t) runs one full 32-k PSUM
            # accumulation group. Pointwise DMA demand stays under the HBM
            # limit so the TensorEngine never starves while x streams in. ----
            w0sb = w0pool.tile([128, KT * OC], dt.bfloat16, tag="w0sb")
            projsb = w0pool.tile([128, KT * 384], dt.float8e4, tag="projsb")
            xsb8 = w0pool.tile([128, KT * 512], dt.float8e4, tag="xsb8")
            xT_r = xT_ap.rearrange("(a p) t -> p a t", p=128)
            wT_r = wT_ap.rearrange("(a p) o -> p a o", p=128)
            projT_r = projT_ap.rearrange("(a p) c -> p a c", p=128)
            xsb_r = xsb.rearrange("p (a t) -> p a t", a=KT)
            w0sb_r = w0sb.rearrange("p (a o) -> p a o", a=KT)
            projsb_r = projsb.rearrange("p (a c) -> p a c", a=KT)
            xR8_r = xR8_ap.rearrange("(a p) t -> p a t", p=128)
            xsb8_r = xsb8.rearrange("p (a t) -> p a t", a=KT)
            ocs0 = slice(0, OC)

            rps = {}
            for h, trange in ((0, range(0, 4)), (1, range(4, 8))):
                # all DMAs for this sweep upfront, in consumption order and
                # batched 4 k-tiles per transfer (~0.6us sync-engine issue
                # cost per DMA caps bandwidth at ~1.6 x size GB/us)
                # startup-tuned issue order: the first k-tiles' x+proj
                # singles go first (router MMs need only those; base MMs are
                # lagged), w0 follows, then 4-tile batches for the rest
                hs = slice(h * 512, (h + 1) * 512)
                if h == 0:
                    # first router pair via idle engines: issues in parallel
                    # with sync's DMA stream right after the preamble barrier
                    nc.gpsimd.dma_start(xsb8_r[:, 0:2, :], xR8_r[:, 0:2, hs])
                    nc.scalar.dma_start(projsb_r[:, 0:2, :], projT_r[:, 0:2, :])
                    for k in range(2, 8, 2):
                        ka = slice(k, k + 2)
                        nc.sync.dma_start(xsb8_r[:, ka, :], xR8_r[:, ka, hs])
                        nc.sync.dma_start(projsb_r[:, ka, :], projT_r[:, ka, :])
                    for k in range(0, 8, 2):
                        ka = slice(k, k + 2)
                        nc.scalar.dma_start(xsb_r[:, ka, hs], xT_r[:, ka, hs])
                        nc.scalar.dma_start(w0sb_r[:, ka, :], wT_r[:, ka, ocs0])
                    for k0 in range(8, KT, 4):
                        ka = slice(k0, k0 + 4)
                        nc.sync.dma_start(xsb8_r[:, ka, :], xR8_r[:, ka, hs])
                        nc.sync.dma_start(projsb_r[:, ka, :], projT_r[:, ka, :])
                        nc.sync.dma_start(xsb_r[:, ka, hs], xT_r[:, ka, hs])
                        nc.sync.dma_start(w0sb_r[:, ka, :], wT_r[:, ka, ocs0])
                else:
                    for k0 in range(0, KT, 4):
                        ka = slice(k0, k0 + 4)
                        nc.sync.dma_start(xsb8_r[:, ka, :], xR8_r[:, ka, hs])
                        nc.sync.dma_start(xsb_r[:, ka, hs], xT_r[:, ka, hs])
                if h == 0:
                    biassb = constp.tile([128, D_OUT], dt.bfloat16, tag="biassb")
                    nc.sync.dma_start(biassb[:], biasr_ap[:])
                else:
                    rwrepsb = constp.tile([128, 128], dt.bfloat16, tag="rwrepsb")
                    nc.sync.dma_start(rwrepsb[:], rwrep_ap[:])
                    bcatsb = constp.tile([128, D_OUT], dt.bfloat16, tag="bcatsb")
                    nc.sync.dma_start(bcatsb[:], bcat_ap[:])

                vps = ps_tile(f"vps{h}")
                ups = ps_tile(f"ups{h}")
                lps = ps_tile(f"lps{h}")
                rps[h] = (vps, ups, lps)
                pst = {t: ps_tile(f"pst0_{t}") for t in trange}
                DELAY = 6 if h == 1 else 8

                def base_mms(k, trange=trange, pst=pst):
                    for t in trange:
                        nc.tensor.matmul(
                            pst[t][:],
                            xsb[:, k * T + t * 128 : k * T + (t + 1) * 128],
                            w0sb[:, k * OC : (k + 1) * OC],
                            start=(k == 0),
                            stop=(k == KT - 1),
                        )

                DR = mybir.MatmulPerfMode.DoubleRow
                for k in range(KT):
                    if k % 2 == 0:
                        kp = k // 2
                        st, sp = kp == 0, kp == KT // 2 - 1
                        kpair = slice(k, k + 2)
                        rx8 = xsb8_r[:, kpair, :]
                        pj = projsb_r[:, kpair, :]
                        nc.tensor.matmul(
                            vps[:], pj[:, :, 0:128], rx8,
                            start=st, stop=sp, perf_mode=DR,
                        )
                        nc.tensor.matmul(
                            ups[:], pj[:, :, 128:256], rx8,
                            start=st, stop=sp, perf_mode=DR,
                        )
                        nc.tensor.matmul(
                            lps[:], pj[:, :, 256:384], rx8,
                            start=st, stop=sp, perf_mode=DR,
                        )
                    if k >= DELAY:
                        base_mms(k - DELAY)
                for k in range(KT - DELAY, KT):
                    base_mms(k)
                for t in trange:
                    a = accpool.tile(
                        [128, 512], dt.float32, tag=f"acc0_{t}", name=f"acc0_{t}"
                    )
                    acc[(0, t)] = a
                    nc.vector.tensor_add(a[:], pst[t][:], biassb[:, ocs0])

                # router epilogue for half h: free the 3 PSUM accumulators
                vtmp = inter.tile([128, 512], dt.float32, tag="vtmp", name=f"vtmp{h}")
                utmp = inter.tile([128, 512], dt.float32, tag="utmp", name=f"utmp{h}")
                nc.scalar.activation(vtmp[:], vps[:], AFT.Tanh, scale=1.0 / 64)
                nc.scalar.activation(utmp[:], ups[:], AFT.Sigmoid, scale=1.0 / 64)
                nc.vector.tensor_mul(vub[:, h * 512 : (h + 1) * 512], vtmp[:], utmp[:])
                if h == 0:
                    nc.vector.tensor_copy(lsb0[:], lps[:])

            # ---- scores + gates + weighted lora (wtb) ----
            lps1 = rps[1][2]
            for h in range(2):
                sl = slice(h * 512, (h + 1) * 512)
                sps = ps_tile(f"sps{h}")
                nc.tensor.matmul(sps[:], rwrepsb[:], vub[:, sl], start=True, stop=True)
                nc.scalar.activation(rwb[:, sl], sps[:], AFT.Sigmoid)
            nc.vector.tensor_mul(wtb[:, 0:512], lsb0[:], rwb[:, 0:512])
            nc.vector.tensor_mul(wtb[:, 512:1024], lps1[:], rwb[:, 512:1024])

            # ---- sweeps 2..7: classic 8-bank PSUM accumulation.
            # The MoE finish matmuls for sweeps 0/1 are interleaved into
            # spare PSUM slots of sweeps 2/3 so they never serialize on
            # DVE bank recycling. ----
            extras = []
            for oc in (0,):
                ocs_f = slice(oc * OC, (oc + 1) * OC)
                for t in range(TT):
                    def emit_finish(oc=oc, t=t, ocs_f=ocs_f):
                        pm = ps_tile(f"pm{oc}_{t}")
                        nc.tensor.matmul(
                            pm[:],
                            wtb[:, t * 128 : (t + 1) * 128],
                            bcatsb[:, ocs_f],
                            start=True,
                            stop=True,
                        )
                        osb = opool.tile(
                            [128, 512], dt.float32, tag="osb", name=f"osbf{oc}_{t}"
                        )
                        nc.vector.tensor_add(osb[:], pm[:], acc[(oc, t)][:])
                        nc.sync.dma_start(
                            out_ap[t * 128 : (t + 1) * 128, ocs_f], osb[:]
                        )
                    extras.append(emit_finish)
            extras_it = iter(extras)

            def classic_sweep(oc, take_extras_kh0, take_extras_kh1):
                ocs = slice(oc * OC, (oc + 1) * OC)
                pst = [None] * TT
                for kh in range(KH):
                    wsb = wpool.tile(
                        [128, KHT * OC], dt.bfloat16, tag="wsb", name=f"wsb{oc}_{kh}"
                    )
                    wsb_r = wsb.rearrange("p (a o) -> p a o", a=KHT)
                    for kk0 in range(0, KHT, 4):
                        nc.sync.dma_start(
                            wsb_r[:, kk0 : kk0 + 4, :],
                            wT_r[:, kh * KHT + kk0 : kh * KHT + kk0 + 4, ocs],
                        )
                    for t in range(TT):
                        if kh == 0:
                            pst[t] = ps_tile(f"pst{oc}_{t}")
                        for kk in range(KHT):
                            k = kh * KHT + kk
                            nc.tensor.matmul(
                                pst[t][:],
                                xsb[:, k * T + t * 128 : k * T + (t + 1) * 128],
                                wsb[:, kk * OC : (kk + 1) * OC],
                                start=(k == 0),
                                stop=False,
                            )
                        if kh == KH - 1:
                            nc.tensor.matmul(
                                pst[t][:],
                                wtb[:, t * 128 : (t + 1) * 128],
                                bcatsb[:, ocs],
                                start=False,
                                stop=True,
                            )
                            osb = opool.tile([128, 512], dt.float32, tag="osb")
                            nc.vector.tensor_add(osb[:], pst[t][:], biassb[:, ocs])
                            if oc == NOC - 1 and t == TT - 1:
                                # final tile: two queues halve the tail DMA
                                o0 = oc * OC
                                nc.sync.dma_start(
                                    out_ap[t * 128 :, o0 : o0 + 256],
                                    osb[:, 0:256],
                                )
                                nc.sync.dma_start(
                                    out_ap[t * 128 :, o0 + 256 : o0 + 512],
                                    osb[:, 256:512],
                                )
                            else:
                                nc.sync.dma_start(
                                    out_ap[t * 128 : (t + 1) * 128, ocs], osb[:]
                                )
                        # interleave a pending finish matmul when a spare
                        # PSUM bank exists (kh0: pst[0..t]+pm <= 8)
                        take = (take_extras_kh1 if kh == KH - 1
                                else (take_extras_kh0 and 1 <= t <= 6))
                        if take:
                            fn = next(extras_it, None)
                            if fn is not None:
                                fn()

            classic_sweep(1, False, True)
            for oc in range(2, NOC):
                classic_sweep(oc, True, True)
            assert next(extras_it, None) is None

    nc.compile()
    _CACHE["nc"] = nc
    return nc


def _prep_in_maps(x, weight, bias, router_V, router_U, router_W, experts_A, experts_B):
    xT_all = np.ascontiguousarray(
        x.reshape(TOKENS, D_IN).T.astype(BF16)
    )  # [D_IN, TOKENS]
    wT = np.ascontiguousarray(weight.T.astype(BF16))  # [D_IN, D_OUT]
    FP8 = ml_dtypes.float8_e4m3
    # projections pre-scaled x64 into fp8's normal range; the x64 is undone
    # by the activation scale (v, u) and by bcat's /64 (lora path)
    projT = np.ascontiguousarray(
        np.concatenate(
            [
                router_V.T,  # [D_IN, 128]
                router_U.T,  # [D_IN, 128]
                experts_A.transpose(1, 0, 2).reshape(D_IN, E * R),  # [D_IN, 128]
            ],
            axis=1,
        )
        * 64.0
    ).astype(FP8)
    xR8_all = xT_all.astype(FP8)
    rwrep = np.ascontiguousarray(np.repeat(router_W, R, axis=0).T.astype(BF16))
    bcat = np.ascontiguousarray((experts_B.reshape(E * R, D_OUT) / 64.0).astype(BF16))
    biasr = np.ascontiguousarray(
        np.broadcast_to(bias.astype(BF16), (128, D_OUT))
    )

    in_maps = []
    for c in range(N_CORES):
        in_maps.append(
            {
                "xT": np.ascontiguousarray(xT_all[:, c * T : (c + 1) * T]),
                "xR8": np.ascontiguousarray(xR8_all[:, c * T : (c + 1) * T]),
                "wT": wT,
                "projT": projT,
                "rwrep": rwrep,
                "bcat": bcat,
                "biasr": biasr,
            }
        )
    return in_maps


def _gather(results):
    out = np.concatenate(
        [np.asarray(results[c]["out"], dtype=np.float32) for c in range(N_CORES)],
        axis=0,
    )
    return out.reshape(B, N, D_OUT)


def kernel(x, weight, bias, router_V, router_U, router_W, experts_A, experts_B):
    import time
    from concourse.bass_utils import run_bass_kernel_spmd

    nc = _get_nc()
    in_maps = _prep_in_maps(
        x, weight, bias, router_V, router_U, router_W, experts_A, experts_B
    )
    last_err = None
    for attempt in range(3):
        try:
            res = run_bass_kernel_spmd(nc, in_maps, list(range(N_CORES)))
            return _gather(res.results)
        except Exception as e:  # transient NRT device errors — retry
            last_err = e
            try:  # drop the (possibly wedged) PJRT device context
                import jax

                jax.clear_caches()
                clear = getattr(
                    getattr(getattr(jax, "extend", None), "backend", None),
                    "clear_backends",
                    None,
                ) or getattr(jax, "clear_backends", None)
                if clear is not None:
                    clear()
            except Exception:
                pass
            time.sleep(5 * (attempt + 1))
    raise last_err


def run_traced(x, weight, bias, router_V, router_U, router_W, experts_A, experts_B):
    """Correctness + HW timing run (profiled). Returns (out, exec_time_ns, trace)."""
    import concourse.bass_utils as bass_utils

    bass_utils.upload_artifacts = lambda tmpdir: tmpdir  # no fileshare here
    nc = _get_nc()
    in_maps = _prep_in_maps(
        x, weight, bias, router_V, router_U, router_W, experts_A, experts_B
    )
    res = bass_utils.run_bass_kernel_spmd(
        nc, in_maps, list(range(N_CORES)), trace=True
    )
    trace_path = None
    if res.instructions_and_trace is not None:
        trace_path = res.instructions_and_trace[1]
    return _gather(res.results), res.exec_time_ns, trace_path



# revision 2
# speedup vs baseline: 1.0690x; 1.0690x over previous
"""Trainium2 Bass kernel for ABMIL-MoE-LoRA linear layer.

Reference computation (B=4, N=2048, D_IN=D_OUT=4096, E=8, R=16, D_ATT=128):
    base = x @ W.T + bias
    v = tanh(x @ V.T); u = sigmoid(x @ U.T)
    rw = sigmoid((v*u) @ router_W.T)                    # [B,N,E]
    lora = x @ A_e  (per expert)                        # [B,N,E,R]
    out = base + sum_e rw[...,e] * (lora_e @ B_e)

Strategy: data-parallel over the B*N = 8192 tokens across 8 NeuronCores
(1024 tokens/core, weights replicated). Matmuls run on the TensorEngine
with fp32 PSUM accumulation. Host-side prep pre-transposes every operand
so the contraction dim lands on SBUF partitions.

Precision split: 26 of the 32 contraction k-tiles of the base matmul run
in bf16; the last 6 run as 3 fp8-e4m3 DoubleRow matmuls (2 k-tiles per
pass, half the PE time). The fp8 operands carry cancelling power-of-2
scales (x/8, W*8) so their partial products accumulate into the SAME
PSUM bank as the bf16 partials with no epilogue fixup. Measured rel err
of the hybrid ~1.7e-2 vs the 2e-2 gate.

Schedule: the router/LoRA-down projections are interleaved into the first
two output-column sweeps (k-tile by k-tile, matching DMA arrival order) so
the TensorEngine never starves while x / weights stream in. Those two
sweeps accumulate base-matmul partials into SBUF (PSUM banks are the
scarce resource); later sweeps use the classic 8-bank PSUM accumulation
with the MoE up-projection matmul fused into the same accumulation group.
A burst of dummy matmuls on a memset tile right after the NEFF preamble
warms the PE HAM clock gate so real matmuls never run at K=4/8.

Self-contained: hardcodes all shapes; only imports installed packages.
"""

import numpy as np
import ml_dtypes

BF16 = ml_dtypes.bfloat16

# Problem shapes (hardcoded per spec)
B, N, D_IN, D_OUT = 4, 2048, 4096, 4096
E, R, D_ATT = 8, 16, 128
TOKENS = B * N            # 8192
N_CORES = 8
T = TOKENS // N_CORES     # 1024 tokens per core
KT = D_IN // 128          # 32 contraction k-tiles
KTB = 26                  # k-tiles 0..25 in bf16
KT8 = KT - KTB            # k-tiles 26..31 in fp8 DoubleRow
KP8 = KT8 // 2            # 3 DoubleRow passes
OC = 512                  # output-column chunk per PSUM bank
NOC = D_OUT // OC         # 8 o-chunks
TT = T // 128             # 8 token tiles per core
KH = 2                    # bf16 weight streamed in 2 k-halves
KHT = KTB // KH           # 13 bf16 k-tiles per half
X8S = 8.0                 # x scaled by 1/X8S, W by X8S for the fp8 split
N_WARMUP = 16             # dummy MMs to warm the PE HAM clock gate

_CACHE = {}


def _get_nc():
    if "nc" in _CACHE:
        return _CACHE["nc"]

    import concourse.tile as tile
    import concourse.mybir as mybir
    from concourse import bacc

    dt = mybir.dt
    AFT = mybir.ActivationFunctionType
    nc = bacc.Bacc("TRN2", target_bir_lowering=False, debug=False)

    xT = nc.declare_dram_parameter("xT", [KTB * 128, T], dt.bfloat16, isOutput=False)
    wT = nc.declare_dram_parameter("wT", [KTB * 128, D_OUT], dt.bfloat16, isOutput=False)
    x8bT = nc.declare_dram_parameter("x8bT", [KT8 * 128, T], dt.float8e4, isOutput=False)
    w8T = nc.declare_dram_parameter("w8T", [KT8 * 128, D_OUT], dt.float8e4, isOutput=False)
    projT = nc.declare_dram_parameter("projT", [D_IN, 384], dt.float8e4, isOutput=False)
    xR8 = nc.declare_dram_parameter("xR8", [D_IN, T], dt.float8e4, isOutput=False)
    rwrep = nc.declare_dram_parameter("rwrep", [128, 128], dt.bfloat16, isOutput=False)
    bcat = nc.declare_dram_parameter("bcat", [E * R, D_OUT], dt.bfloat16, isOutput=False)
    biasr = nc.declare_dram_parameter("biasr", [128, D_OUT], dt.bfloat16, isOutput=False)
    out = nc.declare_dram_parameter("out", [T, D_OUT], dt.float32, isOutput=True)

    xT_ap, wT_ap, projT_ap, xR8_ap = xT.ap(), wT.ap(), projT.ap(), xR8.ap()
    x8bT_ap, w8T_ap = x8bT.ap(), w8T.ap()
    rwrep_ap, bcat_ap, biasr_ap, out_ap = rwrep.ap(), bcat.ap(), biasr.ap(), out.ap()

    with tile.TileContext(nc) as tc:
        with (
            tc.tile_pool(name="xpool", bufs=1) as xpool,
            tc.tile_pool(name="wpool", bufs=2) as wpool,
            tc.tile_pool(name="w8pool", bufs=2) as w8pool,
            tc.tile_pool(name="w0pool", bufs=1) as w0pool,
            tc.tile_pool(name="const", bufs=1) as constp,
            tc.tile_pool(name="inter", bufs=1) as inter,
            tc.tile_pool(name="accpool", bufs=1) as accpool,
            tc.tile_pool(name="opool", bufs=3) as opool,
            tc.tile_pool(name="ps", bufs=8, space="PSUM") as psp,
        ):
            xsb = xpool.tile([128, KTB * T], dt.bfloat16, tag="xsb")
            vub = inter.tile([128, T], dt.bfloat16, tag="vub")
            rwb = inter.tile([128, T], dt.bfloat16, tag="rwb")
            wtb = inter.tile([128, T], dt.bfloat16, tag="wtb")
            lsb0 = inter.tile([128, 512], dt.bfloat16, tag="lsb0")
            acc = {}  # (oc, t) -> SBUF fp32 partial-sum tile for sweeps 0/1

            def ps_tile(name):
                return psp.tile([128, 512], dt.float32, tag="ps", name=name)

            # ---- PE warmup: dummy matmuls on a memset tile so the HAM
            # clock gate reaches K=8/8 before the first data-dependent
            # matmul issues (~12us in, right when the first DMAs land).
            # WAW on the single psum tile keeps them serialized. ----
            wub = constp.tile([128, 512], dt.bfloat16, tag="wub")
            nc.vector.memset(wub[:], 1.0)
            wups = ps_tile("warmup")
            for _ in range(N_WARMUP):
                nc.tensor.matmul(wups[:], wub[:, 0:128], wub[:], start=True, stop=True)

            # ---- sweeps 0 and 1: router half-sweep h fused with the base
            # matmul for o-chunk 0, token-half h. The oc0 weight chunk stays
            # resident across both sweeps; each (t) runs one full PSUM
            # accumulation group (26 bf16 k-tiles + 3 fp8 DoubleRow passes).
            # Pointwise DMA demand stays under the HBM limit so the
            # TensorEngine never starves while x streams in. ----
            w0sb = w0pool.tile([128, KTB * OC], dt.bfloat16, tag="w0sb")
            w8sb0 = w0pool.tile([128, KT8 * OC], dt.float8e4, tag="w8sb0")
            x8b = w0pool.tile([128, KT8 * T], dt.float8e4, tag="x8b")
            projsb = w0pool.tile([128, KT * 384], dt.float8e4, tag="projsb")
            xsb8 = w0pool.tile([128, KT * 512], dt.float8e4, tag="xsb8")
            xT_r = xT_ap.rearrange("(a p) t -> p a t", p=128)
            wT_r = wT_ap.rearrange("(a p) o -> p a o", p=128)
            x8bT_r = x8bT_ap.rearrange("(a p) t -> p a t", p=128)
            w8T_r = w8T_ap.rearrange("(a p) o -> p a o", p=128)
            projT_r = projT_ap.rearrange("(a p) c -> p a c", p=128)
            xsb_r = xsb.rearrange("p (a t) -> p a t", a=KTB)
            w0sb_r = w0sb.rearrange("p (a o) -> p a o", a=KTB)
            w8sb0_r = w8sb0.rearrange("p (a o) -> p a o", a=KT8)
            x8b_r = x8b.rearrange("p (a t) -> p a t", a=KT8)
            projsb_r = projsb.rearrange("p (a c) -> p a c", a=KT)
            xR8_r = xR8_ap.rearrange("(a p) t -> p a t", p=128)
            xsb8_r = xsb8.rearrange("p (a t) -> p a t", a=KT)
            ocs0 = slice(0, OC)

            rps = {}
            for h, trange in ((0, range(0, 4)), (1, range(4, 8))):
                # all DMAs for this sweep upfront, in consumption order and
                # batched 4 k-tiles per transfer (~0.6us sync-engine issue
                # cost per DMA caps bandwidth at ~1.6 x size GB/us)
                # startup-tuned issue order: the first k-tiles' x+proj
                # singles go first (router MMs need only those; base MMs are
                # lagged), w0 follows, then 4-tile batches for the rest
                hs = slice(h * 512, (h + 1) * 512)
                if h == 0:
                    # router-critical first pair on the sync queue (first
                    # DMA slot after the preamble barrier); the second pair
                    # rides the idle gpsimd/scalar queues in parallel
                    nc.sync.dma_start(xsb8_r[:, 0:2, :], xR8_r[:, 0:2, hs])
                    nc.scalar.dma_start(projsb_r[:, 0:2, :], projT_r[:, 0:2, :])
                    nc.gpsimd.dma_start(xsb8_r[:, 2:4, :], xR8_r[:, 2:4, hs])
                    for k in range(4, 8, 2):
                        ka = slice(k, k + 2)
                        nc.sync.dma_start(xsb8_r[:, ka, :], xR8_r[:, ka, hs])
                        nc.sync.dma_start(projsb_r[:, ka, :], projT_r[:, ka, :])
                    nc.scalar.dma_start(projsb_r[:, 2:4, :], projT_r[:, 2:4, :])
                    for k in range(0, 8, 2):
                        ka = slice(k, k + 2)
                        nc.scalar.dma_start(xsb_r[:, ka, hs], xT_r[:, ka, hs])
                        nc.scalar.dma_start(w0sb_r[:, ka, :], wT_r[:, ka, ocs0])
                    for k0 in range(8, KT, 4):
                        ka = slice(k0, k0 + 4)
                        nc.sync.dma_start(xsb8_r[:, ka, :], xR8_r[:, ka, hs])
                        nc.sync.dma_start(projsb_r[:, ka, :], projT_r[:, ka, :])
                        if k0 + 4 <= KTB:
                            kab = ka
                            nc.sync.dma_start(xsb_r[:, kab, hs], xT_r[:, kab, hs])
                            nc.sync.dma_start(w0sb_r[:, kab, :], wT_r[:, kab, ocs0])
                    # bf16 tail k 24..25 + the fp8 base operands (small)
                    ka = slice(24, 26)
                    nc.sync.dma_start(xsb_r[:, ka, hs], xT_r[:, ka, hs])
                    nc.sync.dma_start(w0sb_r[:, ka, :], wT_r[:, ka, ocs0])
                    nc.gpsimd.dma_start(x8b_r[:, :, hs], x8bT_r[:, :, hs])
                    nc.gpsimd.dma_start(w8sb0_r[:, :, :], w8T_r[:, :, ocs0])
                else:
                    nc.gpsimd.dma_start(x8b_r[:, :, hs], x8bT_r[:, :, hs])
                    for k0 in range(0, KT, 4):
                        ka = slice(k0, k0 + 4)
                        nc.sync.dma_start(xsb8_r[:, ka, :], xR8_r[:, ka, hs])
                        if k0 + 4 <= KTB:
                            nc.sync.dma_start(xsb_r[:, ka, hs], xT_r[:, ka, hs])
                    nc.sync.dma_start(xsb_r[:, 24:26, hs], xT_r[:, 24:26, hs])
                if h == 0:
                    biassb = constp.tile([128, D_OUT], dt.bfloat16, tag="biassb")
                    nc.sync.dma_start(biassb[:], biasr_ap[:])
                else:
                    rwrepsb = constp.tile([128, 128], dt.bfloat16, tag="rwrepsb")
                    nc.sync.dma_start(rwrepsb[:], rwrep_ap[:])
                    bcatsb = constp.tile([128, D_OUT], dt.bfloat16, tag="bcatsb")
                    nc.sync.dma_start(bcatsb[:], bcat_ap[:])

                vps = ps_tile(f"vps{h}")
                ups = ps_tile(f"ups{h}")
                lps = ps_tile(f"lps{h}")
                rps[h] = (vps, ups, lps)
                pst = {t: ps_tile(f"pst0_{t}") for t in trange}
                DELAY = 6 if h == 1 else 8

                def base_mms(k, trange=trange, pst=pst):
                    for t in trange:
                        nc.tensor.matmul(
                            pst[t][:],
                            xsb[:, k * T + t * 128 : k * T + (t + 1) * 128],
                            w0sb[:, k * OC : (k + 1) * OC],
                            start=(k == 0),
                            stop=False,
                        )

                DR = mybir.MatmulPerfMode.DoubleRow
                for k in range(KT):
                    if k % 2 == 0:
                        kp = k // 2
                        st, sp = kp == 0, kp == KT // 2 - 1
                        kpair = slice(k, k + 2)
                        rx8 = xsb8_r[:, kpair, :]
                        pj = projsb_r[:, kpair, :]
                        nc.tensor.matmul(
                            vps[:], pj[:, :, 0:128], rx8,
                            start=st, stop=sp, perf_mode=DR,
                        )
                        nc.tensor.matmul(
                            ups[:], pj[:, :, 128:256], rx8,
                            start=st, stop=sp, perf_mode=DR,
                        )
                        nc.tensor.matmul(
                            lps[:], pj[:, :, 256:384], rx8,
                            start=st, stop=sp, perf_mode=DR,
                        )
                    if k >= DELAY and k - DELAY < KTB:
                        base_mms(k - DELAY)
                for k in range(KT - DELAY, KTB):
                    base_mms(k)
                # fp8 DoubleRow tail of each base accumulation group
                for t in trange:
                    for p in range(KP8):
                        pr = slice(2 * p, 2 * p + 2)
                        nc.tensor.matmul(
                            pst[t][:],
                            x8b_r[:, pr, t * 128 : (t + 1) * 128],
                            w8sb0_r[:, pr, :],
                            start=False,
                            stop=(p == KP8 - 1),
                            perf_mode=DR,
                        )
                for t in trange:
                    a = accpool.tile(
                        [128, 512], dt.float32, tag=f"acc0_{t}", name=f"acc0_{t}"
                    )
                    acc[(0, t)] = a
                    nc.vector.tensor_add(a[:], pst[t][:], biassb[:, ocs0])

                # router epilogue for half h: free the 3 PSUM accumulators
                vtmp = inter.tile([128, 512], dt.float32, tag="vtmp", name=f"vtmp{h}")
                utmp = inter.tile([128, 512], dt.float32, tag="utmp", name=f"utmp{h}")
                nc.scalar.activation(vtmp[:], vps[:], AFT.Tanh, scale=1.0 / 64)
                nc.scalar.activation(utmp[:], ups[:], AFT.Sigmoid, scale=1.0 / 64)
                nc.vector.tensor_mul(vub[:, h * 512 : (h + 1) * 512], vtmp[:], utmp[:])
                if h == 0:
                    nc.vector.tensor_copy(lsb0[:], lps[:])

            # ---- scores + gates + weighted lora (wtb) ----
            lps1 = rps[1][2]
            for h in range(2):
                sl = slice(h * 512, (h + 1) * 512)
                sps = ps_tile(f"sps{h}")
                nc.tensor.matmul(sps[:], rwrepsb[:], vub[:, sl], start=True, stop=True)
                nc.scalar.activation(rwb[:, sl], sps[:], AFT.Sigmoid)
            nc.vector.tensor_mul(wtb[:, 0:512], lsb0[:], rwb[:, 0:512])
            nc.vector.tensor_mul(wtb[:, 512:1024], lps1[:], rwb[:, 512:1024])

            # ---- sweeps 2..7: classic 8-bank PSUM accumulation.
            # The MoE finish matmuls for sweeps 0/1 are interleaved into
            # spare PSUM slots of sweeps 2/3 so they never serialize on
            # DVE bank recycling. ----
            extras = []
            for oc in (0,):
                ocs_f = slice(oc * OC, (oc + 1) * OC)
                for t in range(TT):
                    def emit_finish(oc=oc, t=t, ocs_f=ocs_f):
                        pm = ps_tile(f"pm{oc}_{t}")
                        nc.tensor.matmul(
                            pm[:],
                            wtb[:, t * 128 : (t + 1) * 128],
                            bcatsb[:, ocs_f],
                            start=True,
                            stop=True,
                        )
                        osb = opool.tile(
                            [128, 512], dt.float32, tag="osb", name=f"osbf{oc}_{t}"
                        )
                        nc.vector.tensor_add(osb[:], pm[:], acc[(oc, t)][:])
                        nc.sync.dma_start(
                            out_ap[t * 128 : (t + 1) * 128, ocs_f], osb[:]
                        )
                    extras.append(emit_finish)
            extras_it = iter(extras)

            DRm = mybir.MatmulPerfMode.DoubleRow

            def classic_sweep(oc, take_extras_kh0, take_extras_kh1):
                ocs = slice(oc * OC, (oc + 1) * OC)
                pst = [None] * TT
                w8sb = w8pool.tile(
                    [128, KT8 * OC], dt.float8e4, tag="w8sb", name=f"w8sb{oc}"
                )
                w8sb_r = w8sb.rearrange("p (a o) -> p a o", a=KT8)
                nc.sync.dma_start(w8sb_r[:, :, :], w8T_r[:, :, ocs])
                for kh in range(KH):
                    wsb = wpool.tile(
                        [128, KHT * OC], dt.bfloat16, tag="wsb", name=f"wsb{oc}_{kh}"
                    )
                    wsb_r = wsb.rearrange("p (a o) -> p a o", a=KHT)
                    for kk0 in range(0, KHT, 4):
                        kk1 = min(kk0 + 4, KHT)
                        nc.sync.dma_start(
                            wsb_r[:, kk0:kk1, :],
                            wT_r[:, kh * KHT + kk0 : kh * KHT + kk1, ocs],
                        )
                    for t in range(TT):
                        if kh == 0:
                            pst[t] = ps_tile(f"pst{oc}_{t}")
                            # fp8 DoubleRow passes open the group (their
                            # small weight DMA lands before the bf16 bulk)
                            for p in range(KP8):
                                pr = slice(2 * p, 2 * p + 2)
                                nc.tensor.matmul(
                                    pst[t][:],
                                    x8b_r[:, pr, t * 128 : (t + 1) * 128],
                                    w8sb_r[:, pr, :],
                                    start=(p == 0),
                                    stop=False,
                                    perf_mode=DRm,
                                )
                        for kk in range(KHT):
                            k = kh * KHT + kk
                            nc.tensor.matmul(
                                pst[t][:],
                                xsb[:, k * T + t * 128 : k * T + (t + 1) * 128],
                                wsb[:, kk * OC : (kk + 1) * OC],
                                start=False,
                                stop=False,
                            )
                        if kh == KH - 1:
                            nc.tensor.matmul(
                                pst[t][:],
                                wtb[:, t * 128 : (t + 1) * 128],
                                bcatsb[:, ocs],
                                start=False,
                                stop=True,
                            )
                            osb = opool.tile([128, 512], dt.float32, tag="osb")
                            if oc == NOC - 1 and t == TT - 1:
                                # final tile: split add+store across two DMA
                                # queues to halve the tail latency
                                o0 = oc * OC
                                nc.vector.tensor_add(
                                    osb[:, 0:256], pst[t][:, 0:256],
                                    biassb[:, o0 : o0 + 256],
                                )
                                nc.sync.dma_start(
                                    out_ap[t * 128 :, o0 : o0 + 256],
                                    osb[:, 0:256],
                                )
                                nc.vector.tensor_add(
                                    osb[:, 256:512], pst[t][:, 256:512],
                                    biassb[:, o0 + 256 : o0 + 512],
                                )
                                nc.scalar.dma_start(
                                    out_ap[t * 128 :, o0 + 256 : o0 + 512],
                                    osb[:, 256:512],
                                )
                            else:
                                nc.vector.tensor_add(osb[:], pst[t][:], biassb[:, ocs])
                                nc.sync.dma_start(
                                    out_ap[t * 128 : (t + 1) * 128, ocs], osb[:]
                                )
                        # interleave a pending finish matmul when a spare
                        # PSUM bank exists (kh0: pst[0..t]+pm <= 8)
                        take = (take_extras_kh1 if kh == KH - 1
                                else (take_extras_kh0 and 1 <= t <= 6))
                        if take:
                            fn = next(extras_it, None)
                            if fn is not None:
                                fn()

            classic_sweep(1, False, True)
            for oc in range(2, NOC):
                classic_sweep(oc, True, True)
            assert next(extras_it, None) is None

    nc.compile()
    _CACHE["nc"] = nc
    return nc


def _prep_in_maps(x, weight, bias, router_V, router_U, router_W, experts_A, experts_B):
    FP8 = ml_dtypes.float8_e4m3
    xT_full = np.ascontiguousarray(
        x.reshape(TOKENS, D_IN).T.astype(np.float32)
    )  # [D_IN, TOKENS] fp32
    KB = KTB * 128
    xT_all = np.ascontiguousarray(xT_full[:KB].astype(BF16))        # bf16 part
    x8bT_all = np.ascontiguousarray((xT_full[KB:] / X8S).astype(FP8))  # fp8 part
    wT_full = weight.T.astype(np.float32)  # [D_IN, D_OUT]
    wT = np.ascontiguousarray(wT_full[:KB].astype(BF16))
    w8T = np.ascontiguousarray((wT_full[KB:] * X8S).astype(FP8))
    # projections pre-scaled x64 into fp8's normal range; the x64 is undone
    # by the activation scale (v, u) and by bcat's /64 (lora path)
    projT = np.ascontiguousarray(
        np.concatenate(
            [
                router_V.T,  # [D_IN, 128]
                router_U.T,  # [D_IN, 128]
                experts_A.transpose(1, 0, 2).reshape(D_IN, E * R),  # [D_IN, 128]
            ],
            axis=1,
        )
        * 64.0
    ).astype(FP8)
    xR8_all = xT_full.astype(FP8)
    rwrep = np.ascontiguousarray(np.repeat(router_W, R, axis=0).T.astype(BF16))
    bcat = np.ascontiguousarray((experts_B.reshape(E * R, D_OUT) / 64.0).astype(BF16))
    biasr = np.ascontiguousarray(
        np.broadcast_to(bias.astype(BF16), (128, D_OUT))
    )

    in_maps = []
    for c in range(N_CORES):
        ts = slice(c * T, (c + 1) * T)
        in_maps.append(
            {
                "xT": np.ascontiguousarray(xT_all[:, ts]),
                "x8bT": np.ascontiguousarray(x8bT_all[:, ts]),
                "xR8": np.ascontiguousarray(xR8_all[:, ts]),
                "wT": wT,
                "w8T": w8T,
                "projT": projT,
                "rwrep": rwrep,
                "bcat": bcat,
                "biasr": biasr,
            }
        )
    return in_maps


def _gather(results):
    out = np.concatenate(
        [np.asarray(results[c]["out"], dtype=np.float32) for c in range(N_CORES)],
        axis=0,
    )
    return out.reshape(B, N, D_OUT)


def kernel(x, weight, bias, router_V, router_U, router_W, experts_A, experts_B):
    import time
    from concourse.bass_utils import run_bass_kernel_spmd

    nc = _get_nc()
    in_maps = _prep_in_maps(
        x, weight, bias, router_V, router_U, router_W, experts_A, experts_B
    )
    last_err = None
    for attempt in range(3):
        try:
            res = run_bass_kernel_spmd(nc, in_maps, list(range(N_CORES)))
            return _gather(res.results)
        except Exception as e:  # transient NRT device errors — retry
            last_err = e
            try:  # drop the (possibly wedged) PJRT device context
                import jax

                jax.clear_caches()
                clear = getattr(
                    getattr(getattr(jax, "extend", None), "backend", None),
                    "clear_backends",
                    None,
                ) or getattr(jax, "clear_backends", None)
                if clear is not None:
                    clear()
            except Exception:
                pass
            time.sleep(5 * (attempt + 1))
    raise last_err


def run_traced(x, weight, bias, router_V, router_U, router_W, experts_A, experts_B):
    """Correctness + HW timing run (profiled). Returns (out, exec_time_ns, trace)."""
    import concourse.bass_utils as bass_utils

    bass_utils.upload_artifacts = lambda tmpdir: tmpdir  # no fileshare here
    nc = _get_nc()
    in_maps = _prep_in_maps(
        x, weight, bias, router_V, router_U, router_W, experts_A, experts_B
    )
    res = bass_utils.run_bass_kernel_spmd(
        nc, in_maps, list(range(N_CORES)), trace=True
    )
    trace_path = None
    if res.instructions_and_trace is not None:
        trace_path = res.instructions_and_trace[1]
    return _gather(res.results), res.exec_time_ns, trace_path


# revision 6
# speedup vs baseline: 1.1106x; 1.0389x over previous
"""Trainium2 Bass kernel for ABMIL-MoE-LoRA linear layer.

Reference computation (B=4, N=2048, D_IN=D_OUT=4096, E=8, R=16, D_ATT=128):
    base = x @ W.T + bias
    v = tanh(x @ V.T); u = sigmoid(x @ U.T)
    rw = sigmoid((v*u) @ router_W.T)                    # [B,N,E]
    lora = x @ A_e  (per expert)                        # [B,N,E,R]
    out = base + sum_e rw[...,e] * (lora_e @ B_e)

Strategy: data-parallel over the B*N = 8192 tokens across 8 NeuronCores
(1024 tokens/core, weights replicated). Matmuls run on the TensorEngine
with fp32 PSUM accumulation. Host-side prep pre-transposes every operand
so the contraction dim lands on SBUF partitions.

Precision split: 26 of the 32 contraction k-tiles of the base matmul run
in bf16; the last 6 run as 3 fp8-e4m3 DoubleRow matmuls (2 k-tiles per
pass, half the PE time). The fp8 operands carry cancelling power-of-2
scales (x/8, W*8) so their partial products accumulate into the SAME
PSUM bank as the bf16 partials with no epilogue fixup. Measured rel err
of the hybrid ~1.7e-2 vs the 2e-2 gate.

Schedule: the router/LoRA-down projections are interleaved into the first
two output-column sweeps (k-tile by k-tile, matching DMA arrival order) so
the TensorEngine never starves while x / weights stream in. Those two
sweeps accumulate base-matmul partials into SBUF (PSUM banks are the
scarce resource); later sweeps use the classic 8-bank PSUM accumulation
with the MoE up-projection matmul fused into the same accumulation group.
A burst of dummy matmuls on a memset tile right after the NEFF preamble
warms the PE HAM clock gate so real matmuls never run at K=4/8.

Self-contained: hardcodes all shapes; only imports installed packages.
"""

import numpy as np
import ml_dtypes

BF16 = ml_dtypes.bfloat16

# Problem shapes (hardcoded per spec)
B, N, D_IN, D_OUT = 4, 2048, 4096, 4096
E, R, D_ATT = 8, 16, 128
TOKENS = B * N            # 8192
N_CORES = 8
T = TOKENS // N_CORES     # 1024 tokens per core
KT = D_IN // 128          # 32 contraction k-tiles
KTB = 24                  # k-tiles 0..23 in bf16
KT8 = KT - KTB            # k-tiles 24..31 in fp8 DoubleRow
KP8 = KT8 // 2            # 4 DoubleRow passes
OC = 512                  # output-column chunk per PSUM bank
NOC = D_OUT // OC         # 8 o-chunks
TT = T // 128             # 8 token tiles per core
KH = 2                    # bf16 weight streamed in 2 k-halves
KHT = KTB // KH           # 12 bf16 k-tiles per half
X8S = 8.0                 # x scaled by 1/X8S, W by X8S for the fp8 split
N_WARMUP = 10             # dummy MMs to warm the PE HAM clock gate

_CACHE = {}


def _get_nc():
    if "nc" in _CACHE:
        return _CACHE["nc"]

    import concourse.tile as tile
    import concourse.mybir as mybir
    from concourse import bacc

    dt = mybir.dt
    AFT = mybir.ActivationFunctionType
    nc = bacc.Bacc("TRN2", target_bir_lowering=False, debug=False)

    xT = nc.declare_dram_parameter("xT", [KTB * 128, T], dt.bfloat16, isOutput=False)
    wT = nc.declare_dram_parameter("wT", [KTB * 128, D_OUT], dt.bfloat16, isOutput=False)
    x8bT = nc.declare_dram_parameter("x8bT", [KT8 * 128, T], dt.float8e4, isOutput=False)
    w8T = nc.declare_dram_parameter("w8T", [KT8 * 128, D_OUT], dt.float8e4, isOutput=False)
    projT = nc.declare_dram_parameter("projT", [D_IN, 384], dt.float8e4, isOutput=False)
    xR8 = nc.declare_dram_parameter("xR8", [D_IN, T], dt.float8e4, isOutput=False)
    rwrep = nc.declare_dram_parameter("rwrep", [128, 128], dt.bfloat16, isOutput=False)
    bcat = nc.declare_dram_parameter("bcat", [E * R, D_OUT], dt.bfloat16, isOutput=False)
    biasr = nc.declare_dram_parameter("biasr", [128, D_OUT], dt.bfloat16, isOutput=False)
    out = nc.declare_dram_parameter("out", [T, D_OUT], dt.float32, isOutput=True)

    xT_ap, wT_ap, projT_ap, xR8_ap = xT.ap(), wT.ap(), projT.ap(), xR8.ap()
    x8bT_ap, w8T_ap = x8bT.ap(), w8T.ap()
    rwrep_ap, bcat_ap, biasr_ap, out_ap = rwrep.ap(), bcat.ap(), biasr.ap(), out.ap()

    with tile.TileContext(nc) as tc:
        with (
            tc.tile_pool(name="xpool", bufs=1) as xpool,
            tc.tile_pool(name="wpool", bufs=2) as wpool,
            tc.tile_pool(name="w8pool", bufs=2) as w8pool,
            tc.tile_pool(name="w0pool", bufs=1) as w0pool,
            tc.tile_pool(name="const", bufs=1) as constp,
            tc.tile_pool(name="inter", bufs=1) as inter,
            tc.tile_pool(name="accpool", bufs=1) as accpool,
            tc.tile_pool(name="opool", bufs=3) as opool,
            tc.tile_pool(name="ps", bufs=8, space="PSUM") as psp,
        ):
            xsb = xpool.tile([128, KTB * T], dt.bfloat16, tag="xsb")
            vub = inter.tile([128, T], dt.bfloat16, tag="vub")
            rwb = inter.tile([128, T], dt.bfloat16, tag="rwb")
            wtb = inter.tile([128, T], dt.bfloat16, tag="wtb")
            lsb0 = inter.tile([128, 512], dt.bfloat16, tag="lsb0")
            acc = {}  # (oc, t) -> SBUF fp32 partial-sum tile for sweeps 0/1

            def ps_tile(name):
                return psp.tile([128, 512], dt.float32, tag="ps", name=name)

            # ---- PE warmup: dummy matmuls on a memset tile so the HAM
            # clock gate reaches K=8/8 before the first data-dependent
            # matmul issues (~12us in, right when the first DMAs land).
            # WAW on the single psum tile keeps them serialized. ----
            wub = constp.tile([128, 512], dt.bfloat16, tag="wub")
            nc.vector.memset(wub[:], 1.0)
            wups = ps_tile("warmup")
            for _ in range(N_WARMUP):
                nc.tensor.matmul(wups[:], wub[:, 0:128], wub[:], start=True, stop=True)

            # ---- sweeps 0 and 1: router half-sweep h fused with the base
            # matmul for o-chunk 0, token-half h. The oc0 weight chunk stays
            # resident across both sweeps; each (t) runs one full PSUM
            # accumulation group (26 bf16 k-tiles + 3 fp8 DoubleRow passes).
            # Pointwise DMA demand stays under the HBM limit so the
            # TensorEngine never starves while x streams in. ----
            w0sb = w0pool.tile([128, KTB * OC], dt.bfloat16, tag="w0sb")
            w8sb0 = w0pool.tile([128, KT8 * OC], dt.float8e4, tag="w8sb0")
            x8b = w0pool.tile([128, KT8 * T], dt.float8e4, tag="x8b")
            projsb = w0pool.tile([128, KT * 384], dt.float8e4, tag="projsb")
            xsb8 = w0pool.tile([128, KT * 512], dt.float8e4, tag="xsb8")
            xT_r = xT_ap.rearrange("(a p) t -> p a t", p=128)
            wT_r = wT_ap.rearrange("(a p) o -> p a o", p=128)
            x8bT_r = x8bT_ap.rearrange("(a p) t -> p a t", p=128)
            w8T_r = w8T_ap.rearrange("(a p) o -> p a o", p=128)
            projT_r = projT_ap.rearrange("(a p) c -> p a c", p=128)
            xsb_r = xsb.rearrange("p (a t) -> p a t", a=KTB)
            w0sb_r = w0sb.rearrange("p (a o) -> p a o", a=KTB)
            w8sb0_r = w8sb0.rearrange("p (a o) -> p a o", a=KT8)
            x8b_r = x8b.rearrange("p (a t) -> p a t", a=KT8)
            projsb_r = projsb.rearrange("p (a c) -> p a c", a=KT)
            xR8_r = xR8_ap.rearrange("(a p) t -> p a t", p=128)
            xsb8_r = xsb8.rearrange("p (a t) -> p a t", a=KT)
            ocs0 = slice(0, OC)

            rps = {}
            for h, trange in ((0, range(0, 4)), (1, range(4, 8))):
                # all DMAs for this sweep upfront, in consumption order and
                # batched 4 k-tiles per transfer (~0.6us sync-engine issue
                # cost per DMA caps bandwidth at ~1.6 x size GB/us)
                # startup-tuned issue order: the first k-tiles' x+proj
                # singles go first (router MMs need only those; base MMs are
                # lagged), w0 follows, then 4-tile batches for the rest
                hs = slice(h * 512, (h + 1) * 512)
                if h == 0:
                    # router-critical first pair on the sync queue (first
                    # DMA slot after the preamble barrier); the second pair
                    # rides the idle gpsimd/scalar queues in parallel
                    nc.sync.dma_start(xsb8_r[:, 0:2, :], xR8_r[:, 0:2, hs])
                    nc.scalar.dma_start(projsb_r[:, 0:2, :], projT_r[:, 0:2, :])
                    nc.gpsimd.dma_start(xsb8_r[:, 2:4, :], xR8_r[:, 2:4, hs])
                    for k in range(4, 8, 2):
                        ka = slice(k, k + 2)
                        nc.sync.dma_start(xsb8_r[:, ka, :], xR8_r[:, ka, hs])
                        nc.sync.dma_start(projsb_r[:, ka, :], projT_r[:, ka, :])
                    nc.scalar.dma_start(projsb_r[:, 2:4, :], projT_r[:, 2:4, :])
                    for k in range(0, 8, 2):
                        ka = slice(k, k + 2)
                        nc.scalar.dma_start(xsb_r[:, ka, hs], xT_r[:, ka, hs])
                        nc.scalar.dma_start(w0sb_r[:, ka, :], wT_r[:, ka, ocs0])
                    for k0 in range(8, KT, 4):
                        ka = slice(k0, k0 + 4)
                        nc.sync.dma_start(xsb8_r[:, ka, :], xR8_r[:, ka, hs])
                        nc.sync.dma_start(projsb_r[:, ka, :], projT_r[:, ka, :])
                        if k0 + 4 <= KTB:
                            kab = ka
                            nc.sync.dma_start(xsb_r[:, kab, hs], xT_r[:, kab, hs])
                            nc.sync.dma_start(w0sb_r[:, kab, :], wT_r[:, kab, ocs0])
                    # late, non-critical: the fp8 base operands + bias (small,
                    # consumed only by the DR tails / epilogue of this sweep).
                    # Kept at the tail of the sync list so they don't crowd
                    # the bandwidth-saturated router/base stream up front.
                    nc.sync.dma_start(x8b_r[:, :, hs], x8bT_r[:, :, hs])
                    nc.sync.dma_start(w8sb0_r[:, :, :], w8T_r[:, :, ocs0])
                    biassb = constp.tile([128, D_OUT], dt.bfloat16, tag="biassb")
                    nc.sync.dma_start(biassb[:], biasr_ap[:])
                else:
                    for k0 in range(0, KT, 4):
                        ka = slice(k0, k0 + 4)
                        nc.sync.dma_start(xsb8_r[:, ka, :], xR8_r[:, ka, hs])
                        if k0 + 4 <= KTB:
                            nc.sync.dma_start(xsb_r[:, ka, hs], xT_r[:, ka, hs])
                    nc.sync.dma_start(x8b_r[:, :, hs], x8bT_r[:, :, hs])
                    rwrepsb = constp.tile([128, 128], dt.bfloat16, tag="rwrepsb")
                    nc.sync.dma_start(rwrepsb[:], rwrep_ap[:])
                    bcatsb = constp.tile([128, D_OUT], dt.bfloat16, tag="bcatsb")
                    nc.sync.dma_start(bcatsb[:], bcat_ap[:])

                vps = ps_tile(f"vps{h}")
                ups = ps_tile(f"ups{h}")
                lps = ps_tile(f"lps{h}")
                rps[h] = (vps, ups, lps)
                pst = {t: ps_tile(f"pst0_{t}") for t in trange}
                DELAY = 6

                def base_mms(k, trange=trange, pst=pst):
                    for t in trange:
                        nc.tensor.matmul(
                            pst[t][:],
                            xsb[:, k * T + t * 128 : k * T + (t + 1) * 128],
                            w0sb[:, k * OC : (k + 1) * OC],
                            start=(k == 0),
                            stop=False,
                        )

                DR = mybir.MatmulPerfMode.DoubleRow
                for k in range(KT):
                    if k % 2 == 0:
                        kp = k // 2
                        st, sp = kp == 0, kp == KT // 2 - 1
                        kpair = slice(k, k + 2)
                        rx8 = xsb8_r[:, kpair, :]
                        pj = projsb_r[:, kpair, :]
                        nc.tensor.matmul(
                            vps[:], pj[:, :, 0:128], rx8,
                            start=st, stop=sp, perf_mode=DR,
                        )
                        nc.tensor.matmul(
                            ups[:], pj[:, :, 128:256], rx8,
                            start=st, stop=sp, perf_mode=DR,
                        )
                        nc.tensor.matmul(
                            lps[:], pj[:, :, 256:384], rx8,
                            start=st, stop=sp, perf_mode=DR,
                        )
                    if k >= DELAY and k - DELAY < KTB:
                        base_mms(k - DELAY)
                for k in range(KT - DELAY, KTB):
                    base_mms(k)
                # fp8 DoubleRow tail of each base accumulation group
                for t in trange:
                    for p in range(KP8):
                        pr = slice(2 * p, 2 * p + 2)
                        nc.tensor.matmul(
                            pst[t][:],
                            x8b_r[:, pr, t * 128 : (t + 1) * 128],
                            w8sb0_r[:, pr, :],
                            start=False,
                            stop=(p == KP8 - 1),
                            perf_mode=DR,
                        )
                for t in trange:
                    a = accpool.tile(
                        [128, 512], dt.float32, tag=f"acc0_{t}", name=f"acc0_{t}"
                    )
                    acc[(0, t)] = a
                    nc.vector.tensor_add(a[:], pst[t][:], biassb[:, ocs0])

                # router epilogue for half h: free the 3 PSUM accumulators
                vtmp = inter.tile([128, 512], dt.float32, tag="vtmp", name=f"vtmp{h}")
                utmp = inter.tile([128, 512], dt.float32, tag="utmp", name=f"utmp{h}")
                nc.scalar.activation(vtmp[:], vps[:], AFT.Tanh, scale=1.0 / 64)
                nc.scalar.activation(utmp[:], ups[:], AFT.Sigmoid, scale=1.0 / 64)
                nc.vector.tensor_mul(vub[:, h * 512 : (h + 1) * 512], vtmp[:], utmp[:])
                if h == 0:
                    nc.vector.tensor_copy(lsb0[:], lps[:])

            # ---- scores + gates + weighted lora (wtb) ----
            lps1 = rps[1][2]
            for h in range(2):
                sl = slice(h * 512, (h + 1) * 512)
                sps = ps_tile(f"sps{h}")
                nc.tensor.matmul(sps[:], rwrepsb[:], vub[:, sl], start=True, stop=True)
                nc.scalar.activation(rwb[:, sl], sps[:], AFT.Sigmoid)
            nc.vector.tensor_mul(wtb[:, 0:512], lsb0[:], rwb[:, 0:512])
            nc.vector.tensor_mul(wtb[:, 512:1024], lps1[:], rwb[:, 512:1024])

            # ---- sweeps 2..7: classic 8-bank PSUM accumulation.
            # The MoE finish matmuls for sweeps 0/1 are interleaved into
            # spare PSUM slots of sweeps 2/3 so they never serialize on
            # DVE bank recycling. ----
            extras = []
            for oc in (0,):
                ocs_f = slice(oc * OC, (oc + 1) * OC)
                for t in range(TT):
                    def emit_finish(oc=oc, t=t, ocs_f=ocs_f):
                        pm = ps_tile(f"pm{oc}_{t}")
                        nc.tensor.matmul(
                            pm[:],
                            wtb[:, t * 128 : (t + 1) * 128],
                            bcatsb[:, ocs_f],
                            start=True,
                            stop=True,
                        )
                        osb = opool.tile(
                            [128, 512], dt.float32, tag="osb", name=f"osbf{oc}_{t}"
                        )
                        nc.vector.tensor_add(osb[:], pm[:], acc[(oc, t)][:])
                        nc.sync.dma_start(
                            out_ap[t * 128 : (t + 1) * 128, ocs_f], osb[:]
                        )
                    extras.append(emit_finish)
            extras_it = iter(extras)

            DRm = mybir.MatmulPerfMode.DoubleRow

            def classic_sweep(oc, take_extras_kh0, take_extras_kh1):
                ocs = slice(oc * OC, (oc + 1) * OC)
                pst = [None] * TT
                w8sb = w8pool.tile(
                    [128, KT8 * OC], dt.float8e4, tag="w8sb", name=f"w8sb{oc}"
                )
                w8sb_r = w8sb.rearrange("p (a o) -> p a o", a=KT8)
                nc.sync.dma_start(w8sb_r[:, :, :], w8T_r[:, :, ocs])
                for kh in range(KH):
                    wsb = wpool.tile(
                        [128, KHT * OC], dt.bfloat16, tag="wsb", name=f"wsb{oc}_{kh}"
                    )
                    wsb_r = wsb.rearrange("p (a o) -> p a o", a=KHT)
                    for kk0 in range(0, KHT, 4):
                        kk1 = min(kk0 + 4, KHT)
                        nc.sync.dma_start(
                            wsb_r[:, kk0:kk1, :],
                            wT_r[:, kh * KHT + kk0 : kh * KHT + kk1, ocs],
                        )
                    for t in range(TT):
                        if kh == 0:
                            pst[t] = ps_tile(f"pst{oc}_{t}")
                            # fp8 DoubleRow passes open the group (their
                            # small weight DMA lands before the bf16 bulk)
                            for p in range(KP8):
                                pr = slice(2 * p, 2 * p + 2)
                                nc.tensor.matmul(
                                    pst[t][:],
                                    x8b_r[:, pr, t * 128 : (t + 1) * 128],
                                    w8sb_r[:, pr, :],
                                    start=(p == 0),
                                    stop=False,
                                    perf_mode=DRm,
                                )
                        for kk in range(KHT):
                            k = kh * KHT + kk
                            nc.tensor.matmul(
                                pst[t][:],
                                xsb[:, k * T + t * 128 : k * T + (t + 1) * 128],
                                wsb[:, kk * OC : (kk + 1) * OC],
                                start=False,
                                stop=False,
                            )
                        if kh == KH - 1:
                            nc.tensor.matmul(
                                pst[t][:],
                                wtb[:, t * 128 : (t + 1) * 128],
                                bcatsb[:, ocs],
                                start=False,
                                stop=True,
                            )
                            osb = opool.tile([128, 512], dt.float32, tag="osb")
                            if oc == NOC - 1 and t == TT - 1:
                                # final tile: split add+store across two DMA
                                # queues to halve the tail latency
                                o0 = oc * OC
                                nc.vector.tensor_add(
                                    osb[:, 0:256], pst[t][:, 0:256],
                                    biassb[:, o0 : o0 + 256],
                                )
                                nc.sync.dma_start(
                                    out_ap[t * 128 :, o0 : o0 + 256],
                                    osb[:, 0:256],
                                )
                                nc.vector.tensor_add(
                                    osb[:, 256:512], pst[t][:, 256:512],
                                    biassb[:, o0 + 256 : o0 + 512],
                                )
                                nc.scalar.dma_start(
                                    out_ap[t * 128 :, o0 + 256 : o0 + 512],
                                    osb[:, 256:512],
                                )
                            else:
                                nc.vector.tensor_add(osb[:], pst[t][:], biassb[:, ocs])
                                nc.sync.dma_start(
                                    out_ap[t * 128 : (t + 1) * 128, ocs], osb[:]
                                )
                        # interleave a pending finish matmul when a spare
                        # PSUM bank exists (kh0: pst[0..t]+pm <= 8)
                        take = (take_extras_kh1 if kh == KH - 1
                                else (take_extras_kh0 and 1 <= t <= 6))
                        if take:
                            fn = next(extras_it, None)
                            if fn is not None:
                                fn()

            classic_sweep(1, False, True)
            for oc in range(2, NOC):
                classic_sweep(oc, True, True)
            assert next(extras_it, None) is None

    nc.compile()
    _CACHE["nc"] = nc
    return nc


def _prep_in_maps(x, weight, bias, router_V, router_U, router_W, experts_A, experts_B):
    FP8 = ml_dtypes.float8_e4m3
    xT_full = np.ascontiguousarray(
        x.reshape(TOKENS, D_IN).T.astype(np.float32)
    )  # [D_IN, TOKENS] fp32
    KB = KTB * 128
    xT_all = np.ascontiguousarray(xT_full[:KB].astype(BF16))        # bf16 part
    x8bT_all = np.ascontiguousarray((xT_full[KB:] / X8S).astype(FP8))  # fp8 part
    wT_full = weight.T.astype(np.float32)  # [D_IN, D_OUT]
    wT = np.ascontiguousarray(wT_full[:KB].astype(BF16))
    w8T = np.ascontiguousarray((wT_full[KB:] * X8S).astype(FP8))
    # projections pre-scaled x64 into fp8's normal range; the x64 is undone
    # by the activation scale (v, u) and by bcat's /64 (lora path)
    projT = np.ascontiguousarray(
        np.concatenate(
            [
                router_V.T,  # [D_IN, 128]
                router_U.T,  # [D_IN, 128]
                experts_A.transpose(1, 0, 2).reshape(D_IN, E * R),  # [D_IN, 128]
            ],
            axis=1,
        )
        * 64.0
    ).astype(FP8)
    xR8_all = xT_full.astype(FP8)
    rwrep = np.ascontiguousarray(np.repeat(router_W, R, axis=0).T.astype(BF16))
    bcat = np.ascontiguousarray((experts_B.reshape(E * R, D_OUT) / 64.0).astype(BF16))
    biasr = np.ascontiguousarray(
        np.broadcast_to(bias.astype(BF16), (128, D_OUT))
    )

    in_maps = []
    for c in range(N_CORES):
        ts = slice(c * T, (c + 1) * T)
        in_maps.append(
            {
                "xT": np.ascontiguousarray(xT_all[:, ts]),
                "x8bT": np.ascontiguousarray(x8bT_all[:, ts]),
                "xR8": np.ascontiguousarray(xR8_all[:, ts]),
                "wT": wT,
                "w8T": w8T,
                "projT": projT,
                "rwrep": rwrep,
                "bcat": bcat,
                "biasr": biasr,
            }
        )
    return in_maps


def _gather(results):
    out = np.concatenate(
        [np.asarray(results[c]["out"], dtype=np.float32) for c in range(N_CORES)],
        axis=0,
    )
    return out.reshape(B, N, D_OUT)


def kernel(x, weight, bias, router_V, router_U, router_W, experts_A, experts_B):
    import time
    from concourse.bass_utils import run_bass_kernel_spmd

    nc = _get_nc()
    in_maps = _prep_in_maps(
        x, weight, bias, router_V, router_U, router_W, experts_A, experts_B
    )
    last_err = None
    for attempt in range(3):
        try:
            res = run_bass_kernel_spmd(nc, in_maps, list(range(N_CORES)))
            return _gather(res.results)
        except Exception as e:  # transient NRT device errors — retry
            last_err = e
            try:  # drop the (possibly wedged) PJRT device context
                import jax

                jax.clear_caches()
                clear = getattr(
                    getattr(getattr(jax, "extend", None), "backend", None),
                    "clear_backends",
                    None,
                ) or getattr(jax, "clear_backends", None)
                if clear is not None:
                    clear()
            except Exception:
                pass
            time.sleep(5 * (attempt + 1))
    raise last_err


def run_traced(x, weight, bias, router_V, router_U, router_W, experts_A, experts_B):
    """Correctness + HW timing run (profiled). Returns (out, exec_time_ns, trace)."""
    import concourse.bass_utils as bass_utils

    bass_utils.upload_artifacts = lambda tmpdir: tmpdir  # no fileshare here
    nc = _get_nc()
    in_maps = _prep_in_maps(
        x, weight, bias, router_V, router_U, router_W, experts_A, experts_B
    )
    res = bass_utils.run_bass_kernel_spmd(
        nc, in_maps, list(range(N_CORES)), trace=True
    )
    trace_path = None
    if res.instructions_and_trace is not None:
        trace_path = res.instructions_and_trace[1]
    return _gather(res.results), res.exec_time_ns, trace_path
